# revision 1
# baseline (speedup 1.0000x reference)
"""Trainium2 Bass kernel for strict-causal (pixelSNAIL) attention.

Problem: B=8, H=W=64 (N=4096), Ck=64, Cv=128, fp32.
    out[b] = softmax(mask(q@k^T/sqrt(Ck))) @ v   with strictly-causal mask
    (pixel i attends only to j < i; row 0 gets all-zero output).

Sharding: data-parallel over batch — one batch per NeuronCore, 8 cores.

Per-core algorithm (flash-like, but full row extents fit on chip):
  - PE-transpose q,k -> qT,kT [64, 4096] (fp32r) so scores matmuls contract
    over the channel dim on partitions.
  - For each q-chunk of 512 rows (4 q-tiles of 128):
      S[128q, k..] = qT_i^T @ kT  (fp32r matmuls, PSUM, causal extent only)
      diagonal 128x128 block gets a -1e9 strict-upper bias (DVE add)
      P = exp(0.125*S)  on ScalarE, PSUM->SBUF bf16, accum_out = row sums
      P_T tiles via PE transpose (bf16) -> PSUM -> DVE copy -> SBUF
      O^T[128v, 512q] += V_j^T @ P_T_j  (bf16 matmuls, PSUM accumulate)
      O^T -> SBUF -> PE transpose -> O[128q, 128v], normalized by 1/rowsum
      (DVE tensor_scalar on the PSUM->SBUF copy), DMA out.
"""

import os
import sys

sys.path.insert(0, "/opt/trn_rl_repo")

import numpy as np

import concourse.bass as bass
import concourse.bacc as bacc
import concourse.mybir as mybir
import concourse.tile as tile
from concourse.bass_utils import run_bass_kernel_spmd
from concourse.masks import make_identity

F32 = mybir.dt.float32
F32R = mybir.dt.float32r
BF16 = mybir.dt.bfloat16

B, H, W, CK, CV = 8, 64, 64, 64, 128
N = H * W            # 4096
NT = N // 128        # 32 q-tiles / k-tiles
NCHUNK = N // 512    # 8 q-chunks
NEG = 1e9
SCALE = 1.0 / np.sqrt(CK)


def build_kernel(repeats=1):
    nc = bacc.Bacc("TRN2", target_bir_lowering=False, debug=False, num_devices=8)

    q = nc.dram_tensor("q", [N, CK], F32, kind="ExternalInput").ap()
    k = nc.dram_tensor("k", [N, CK], F32, kind="ExternalInput").ap()
    v = nc.dram_tensor("v", [N, CV], F32, kind="ExternalInput").ap()
    o = nc.dram_tensor("o", [N, CV], F32, kind="ExternalOutput").ap()

    with tile.TileContext(nc) as tc:
        with (
            tc.tile_pool(name="const", bufs=1) as const_pool,
            tc.tile_pool(name="stage", bufs=1) as stage_pool,
            tc.tile_pool(name="qkT", bufs=1) as qkt_pool,
            tc.tile_pool(name="vsb", bufs=1) as v_pool,
            tc.tile_pool(name="p", bufs=3) as p_pool,
            tc.tile_pool(name="pt", bufs=6) as pt_pool,
            tc.tile_pool(name="osb", bufs=6) as o_pool,
            tc.tile_pool(name="stats", bufs=8) as stats_pool,
            tc.tile_pool(name="ps_s", bufs=2, space="PSUM") as ps_s,
            tc.tile_pool(name="ps_pt", bufs=2, space="PSUM") as ps_pt,
            tc.tile_pool(name="ps_ot", bufs=2, space="PSUM") as ps_ot,
        ):
            def emit_body():
                # ---- constants ----
                ident = const_pool.tile([128, 128], F32)
                make_identity(nc, ident[:])
                ident_bf = const_pool.tile([128, 128], BF16)
                nc.vector.tensor_copy(ident_bf[:], ident[:])

                # strict-causal bias as a matmul operand: tri[c, q] = -NEG
                # where c >= q, so tri^T @ I adds -NEG at [q, k] for k >= q.
                # Accumulating it into the scores PSUM group keeps masking on
                # PE, off the DVE->exp critical chain.
                tri_bf = const_pool.tile([128, 128], BF16)
                nc.gpsimd.memset(tri_bf[:], 0.0)
                nc.gpsimd.affine_select(
                    out=tri_bf[:],
                    in_=tri_bf[:],
                    compare_op=mybir.AluOpType.is_gt,  # keep 0 where q - c > 0
                    fill=-NEG,
                    base=0,
                    pattern=[[1, 128]],
                    channel_multiplier=-1,
                )

                # ---- load & transpose q, k -> qT, kT [64, N] fp32r ----
                qT = qkt_pool.tile([64, N], F32R, tag="qT")
                kT = qkt_pool.tile([64, N], F32R, tag="kT")
                q_stg = stage_pool.tile([128, NT, CK], F32, tag="q_stage")
                k_stg = stage_pool.tile([128, NT, CK], F32, tag="k_stage")
                v_bf = v_pool.tile([128, NT, CV], BF16)
                vstg = stage_pool.tile([128, NT, CV], F32, tag="v_stage")

                q_r = q.rearrange("(t p) c -> p t c", p=128)
                k_r = k.rearrange("(t p) c -> p t c", p=128)
                v_r = v.rearrange("(t p) c -> p t c", p=128)
                for d in range(8):
                    nc.sync.dma_start(
                        q_stg[:, 4 * d : 4 * (d + 1), :],
                        q_r[:, 4 * d : 4 * (d + 1), :],
                    )
                    nc.sync.dma_start(
                        k_stg[:, 4 * d : 4 * (d + 1), :],
                        k_r[:, 4 * d : 4 * (d + 1), :],
                    )
                for d in range(4):
                    nc.sync.dma_start(
                        vstg[:, 8 * d : 8 * (d + 1), :],
                        v_r[:, 8 * d : 8 * (d + 1), :],
                    )
                    nc.vector.tensor_copy(
                        v_bf[:, 8 * d : 8 * (d + 1), :],
                        vstg[:, 8 * d : 8 * (d + 1), :],
                    )

                def make_qkt(g, stg, dst):
                    def emit():
                        ptr = ps_pt.tile([64, 512], F32, tag="ptr", name="ptr")
                        for u in range(4):
                            t = 4 * g + u
                            nc.tensor.transpose(
                                ptr[:, u * 128 : (u + 1) * 128],
                                stg[:, t, :],
                                ident[:],
                            )
                        nc.vector.tensor_copy(
                            dst[:, g * 512 : (g + 1) * 512], ptr[:]
                        )

                    return emit

                # group 0 of q and k inline (chunk 0 needs them)
                make_qkt(0, q_stg, qT)()
                make_qkt(0, k_stg, kT)()
                qk_pending = [
                    make_qkt(g, stg, dst)
                    for g in range(1, NT // 4)
                    for stg, dst in ((q_stg, qT), (k_stg, kT))
                ]
                qk_done = [0]  # highest group fully flushed

                def flush_qk(up_to_group):
                    while qk_done[0] < up_to_group and qk_pending:
                        qk_pending.pop(0)()
                        qk_pending.pop(0)()
                        qk_done[0] += 1

                # ---- main loop over q-chunks ----
                # Rounds of 1024 k-columns (2 PSUM banks / 8 k-tiles) pipelined:
                # PE transpose+PV work for round g-1 is interleaved between the
                # score matmuls of round g so PE never starves behind ScalarE.
                pending = []  # deferred transpose+copy+PV emitters, j order
                carry_pv = [None]  # PV emitter for the group one behind

                def flush(nmax=None):
                    nwork = len(pending) if nmax is None else min(nmax, len(pending))
                    for w in pending[:nwork]:
                        w()
                    del pending[:nwork]

                for c in range(NCHUNK):
                    flush_qk(min(c + 1, NT // 4 - 1))
                    p_tiles = []
                    recips = []
                    accs = []
                    for t in range(4):
                        p_tile = p_pool.tile([128, N], BF16, tag=f"p{t}", name=f"p{t}")
                        p_tiles.append(p_tile)
                        acc = stats_pool.tile([128, 4], F32, tag=f"acc{t}", name=f"acc{t}")
                        accs.append(acc)
                        recip = stats_pool.tile(
                            [128, 1], F32, tag=f"recip{t}", name=f"recip{t}"
                        )
                        recips.append(recip)
                    ot_ps = ps_ot.tile([128, 512], F32, tag="ot")
                    ngroups = (c + 2) // 2  # ceil((4c+4)/8)

                    def make_tpv(c, j0, njs, first, last, ot_ps=ot_ps,
                                 p_tiles=p_tiles):
                        def emit():
                            pt_ps = ps_pt.tile([128, 1024], BF16, tag="ptr",
                                               name="pt_ps")
                            lo = 1024
                            for u in range(njs):
                                j = j0 + u
                                t0 = max(0, j - 4 * c)
                                lo = min(lo, 512 * u + 128 * t0)
                                for t in range(t0, 4):
                                    nc.tensor.transpose(
                                        pt_ps[:, 512 * u + 128 * t : 512 * u + 128 * (t + 1)],
                                        p_tiles[t][:, j * 128 : (j + 1) * 128],
                                        ident_bf[:],
                                    )
                            pt_sb = pt_pool.tile([128, 1024], BF16, tag="pt_sb",
                                                 name="pt_sb")
                            nc.vector.tensor_copy(pt_sb[:, lo:], pt_ps[:, lo:])
                            # run previous group's PV now (pipelined one behind)
                            if carry_pv[0] is not None:
                                carry_pv[0]()

                            def pv():
                                for u in range(njs):
                                    j = j0 + u
                                    qs = 512 * u + 128 * max(0, j - 4 * c)
                                    nc.tensor.matmul(
                                        ot_ps[:, qs - 512 * u :],
                                        v_bf[:, j, :],
                                        pt_sb[:, qs : 512 * (u + 1)],
                                        start=(j == 0 and first),
                                        stop=(j == j0 + njs - 1 and last),
                                    )

                            carry_pv[0] = pv

                        return emit

                    def make_tail(c, ot_ps=ot_ps, recips=recips):
                        def emit():
                            # flush the final PV group of this chunk
                            carry_pv[0]()
                            carry_pv[0] = None
                            ot_sb = o_pool.tile([128, 512], F32, tag="ot_sb",
                                                name="ot_sb")
                            nc.vector.tensor_copy(ot_sb[:], ot_ps[:])
                            otr_ps = ps_pt.tile([128, 512], F32, tag="ptr",
                                                name="otr_ps")
                            for t in range(4):
                                nc.tensor.transpose(
                                    otr_ps[:, t * 128 : (t + 1) * 128],
                                    ot_sb[:, t * 128 : (t + 1) * 128],
                                    ident[:],
                                )
                            o_sb = o_pool.tile([128, 4, CV], F32, tag="o_sb",
                                               name="o_sb")
                            for t in range(4):
                                nc.vector.tensor_scalar_mul(
                                    o_sb[:, t, :],
                                    otr_ps[:, t * 128 : (t + 1) * 128],
                                    recips[t][:],
                                )
                            nc.sync.dma_start(
                                o[512 * c : 512 * (c + 1), :].rearrange(
                                    "(t p) c -> p t c", p=128
                                ),
                                o_sb[:],
                            )

                        return emit

                    for g in range(ngroups):
                        for t in range(4):
                            i = 4 * c + t
                            if i < 8 * g:
                                continue
                            span = 128 * (i + 1)
                            k0 = 1024 * g
                            cols = min(1024, span - k0)
                            s_ps = ps_s.tile([128, 1024], F32, tag="s")
                            d0 = 128 * (i % 8) if g == i // 8 else -1
                            for sub in (0, 512):
                                sc = cols - sub
                                if sc <= 0:
                                    break
                                mc = max(256, min(512, sc))
                                diag_here = 0 <= d0 - sub < 512 and d0 < cols
                                nc.tensor.matmul(
                                    s_ps[:, sub : sub + mc],
                                    qT[:, i * 128 : (i + 1) * 128],
                                    kT[:, k0 + sub : k0 + sub + mc],
                                    start=True,
                                    stop=not diag_here,
                                )
                                if diag_here:
                                    nc.tensor.matmul(
                                        s_ps[:, d0 : d0 + 128],
                                        tri_bf[:],
                                        ident_bf[:],
                                        start=False,
                                        stop=True,
                                    )
                            nc.scalar.activation(
                                p_tiles[t][:, k0 : k0 + cols],
                                s_ps[:, :cols],
                                mybir.ActivationFunctionType.Exp,
                                scale=SCALE,
                                accum_out=accs[t][:, g : g + 1],
                            )
                            if g == i // 8:
                                # this tile's last group: finalize 1/rowsum
                                ssum = stats_pool.tile([128, 1], F32, tag="ssum")
                                if g > 0:
                                    nc.vector.reduce_sum(
                                        ssum[:],
                                        accs[t][:, : g + 1],
                                        axis=mybir.AxisListType.X,
                                    )
                                else:
                                    nc.vector.tensor_copy(ssum[:], accs[t][:, :1])
                                nc.vector.tensor_scalar_add(ssum[:], ssum[:], 1e-30)
                                nc.vector.reciprocal(recips[t][:], ssum[:])
                            # interleave deferred transpose+PV work on PE
                            flush(1)
                        # queue transpose+PV work for this round's k-tiles
                        j_lo = 8 * g
                        j_hi = min(8 * g + 8, 4 * c + 4)
                        for j0 in range(j_lo, j_hi, 2):
                            pending.append(
                                make_tpv(
                                    c,
                                    j0,
                                    2,
                                    first=(j0 == 0),
                                    last=(j0 + 2 >= 4 * c + 4),
                                )
                            )
                    pending.append(make_tail(c))

                flush()

            if repeats > 1:
                with tc.For_i(0, repeats, 1):
                    emit_body()
            else:
                emit_body()

    nc.compile()
    return nc


_NC_CACHE = None


def kernel(**inputs: np.ndarray) -> np.ndarray:
    global _NC_CACHE
    if _NC_CACHE is None:
        _NC_CACHE = build_kernel()
    nc = _NC_CACHE

    query = np.ascontiguousarray(inputs["query"], dtype=np.float32)
    key = np.ascontiguousarray(inputs["key"], dtype=np.float32)
    value = np.ascontiguousarray(inputs["value"], dtype=np.float32)

    in_maps = [
        {
            "q": query[b].reshape(N, CK),
            "k": key[b].reshape(N, CK),
            "v": value[b].reshape(N, CV),
        }
        for b in range(B)
    ]
    res = run_bass_kernel_spmd(nc, in_maps, list(range(B)))
    out = np.stack([res.results[b]["o"] for b in range(B)], axis=0)
    return out.reshape(B, H, W, CV)


def run_traced(inputs_np):
    """Run with NTFF tracing, return HW exec time in ns (max over cores)."""
    global _NC_CACHE
    if _NC_CACHE is None:
        _NC_CACHE = build_kernel()
    nc = _NC_CACHE
    query = inputs_np["query"].reshape(B, N, CK)
    key = inputs_np["key"].reshape(B, N, CK)
    value = inputs_np["value"].reshape(B, N, CV)
    in_maps = [
        {"q": query[b], "k": key[b], "v": value[b]} for b in range(B)
    ]
    res = run_bass_kernel_spmd(nc, in_maps, list(range(B)), trace=True)
    return res.exec_time_ns


if __name__ == "__main__":
    rng = np.random.default_rng(0)
    qq = rng.standard_normal((B, H, W, CK), dtype=np.float32)
    kk = rng.standard_normal((B, H, W, CK), dtype=np.float32)
    vv = rng.standard_normal((B, H, W, CV), dtype=np.float32)
    out = kernel(query=qq, key=kk, value=vv)
    print("out", out.shape, out.dtype, np.abs(out).mean())



# revision 13
# speedup vs baseline: 1.7209x; 1.7209x over previous
"""Trainium2 Bass kernel for strict-causal (pixelSNAIL) attention.

Problem: B=8, H=W=64 (N=4096), Ck=64, Cv=128, fp32.
    out[b] = softmax(mask(q@k^T/sqrt(Ck))) @ v   with strictly-causal mask
    (pixel i attends only to j < i; row 0 gets all-zero output).

Sharding: data-parallel over batch - one batch per NeuronCore, 8 cores.

Per-core algorithm (v2: transposed-score layout, no P transposes):
  - PE-transpose q,k -> qT,kT [64, 4096] (f32r) so matmuls contract over
    the channel dim on partitions.
  - For each q-chunk of 512 rows, loop over k-tiles j (causal extent),
    two j per PSUM tile:
      S^T[128k, q..] = kT_j^T @ qT_chunk   (f32r matmul, PSUM)
      P^T = exp(0.125*S^T)  ScalarE, PSUM->SBUF bf16 (valid region only)
      diagonal k-tile: strict-causal zeroing of P^T via DVE affine_select
      O[128q, 129] += P^T_{j,i}^T @ [V_j | 1]  (bf16 matmul per q-tile i,
         PSUM accumulate over j; col 128 accumulates the softmax rowsum)
  - Normalize on DVE: recip = 1/(rowsum+eps); o_sb = O * recip; DMA out.
    Output lands directly in [q, v] layout - no output transposes.
"""

import os
import sys

sys.path.insert(0, "/opt/trn_rl_repo")

import numpy as np

import concourse.bass as bass
import concourse.bacc as bacc
import concourse.mybir as mybir
import concourse.tile as tile
from concourse.bass_utils import run_bass_kernel_spmd
from concourse.masks import make_identity

F32 = mybir.dt.float32
F32R = mybir.dt.float32r
BF16 = mybir.dt.bfloat16

B, H, W, CK, CV = 8, 64, 64, 64, 128
N = H * W            # 4096
NT = N // 128        # 32 q-tiles / k-tiles
NCHUNK = N // 512    # 8 q-chunks
SCALE = 1.0 / np.sqrt(CK)


def build_kernel(repeats=1):
    nc = bacc.Bacc("TRN2", target_bir_lowering=False, debug=False, num_devices=8)

    q = nc.dram_tensor("q", [N, CK], F32, kind="ExternalInput").ap()
    k = nc.dram_tensor("k", [N, CK], F32, kind="ExternalInput").ap()
    v = nc.dram_tensor("v", [N, CV], F32, kind="ExternalInput").ap()
    o = nc.dram_tensor("o", [N, CV], F32, kind="ExternalOutput").ap()

    with tile.TileContext(nc) as tc:
        with (
            tc.tile_pool(name="const", bufs=1) as const_pool,
            tc.tile_pool(name="stage", bufs=1) as stage_pool,
            tc.tile_pool(name="qkT", bufs=1) as qkt_pool,
            tc.tile_pool(name="vsb", bufs=1) as v_pool,
            tc.tile_pool(name="p", bufs=3) as p_pool,
            tc.tile_pool(name="osb", bufs=8) as o_pool,
            tc.tile_pool(name="stats", bufs=8) as stats_pool,
            tc.tile_pool(name="ps_s", bufs=2, space="PSUM") as ps_s,
            tc.tile_pool(name="ps_o", bufs=2, space="PSUM") as ps_o,
        ):
            def emit_body():
                # ---- constants ----
                ident = const_pool.tile([128, 128], F32)
                make_identity(nc, ident[:])
                # strict-causal keep-mask for diagonal tiles of P^T[k, q]:
                # 1.0 where k < q (partition < column), else 0.0
                mask_bf = const_pool.tile([128, 128], BF16)
                nc.gpsimd.memset(mask_bf[:], 1.0)
                nc.gpsimd.affine_select(
                    out=mask_bf[:],
                    in_=mask_bf[:],
                    compare_op=mybir.AluOpType.is_gt,  # keep 1 where q - k > 0
                    fill=0.0,
                    base=0,
                    pattern=[[1, 128]],
                    channel_multiplier=-1,
                )

                # ---- load q, k, v; build v_aug = [V | 1] in bf16 ----
                q_stg = stage_pool.tile([128, NT, CK], F32, tag="q_stage")
                k_stg = stage_pool.tile([128, NT, CK], F32, tag="k_stage")
                vstg = stage_pool.tile([128, NT, CV], F32, tag="v_stage")
                v_aug = v_pool.tile([128, NT, CV + 1], BF16)

                nc.vector.memset(v_aug[:, :, CV], 1.0)

                q_r = q.rearrange("(t p) c -> p t c", p=128)
                k_r = k.rearrange("(t p) c -> p t c", p=128)
                v_r = v.rearrange("(t p) c -> p t c", p=128)
                for d in range(8):
                    sl = slice(4 * d, 4 * (d + 1))
                    nc.sync.dma_start(q_stg[:, sl, :], q_r[:, sl, :])
                    nc.sync.dma_start(k_stg[:, sl, :], k_r[:, sl, :])
                    nc.sync.dma_start(vstg[:, sl, :], v_r[:, sl, :])
                    nc.vector.tensor_copy(v_aug[:, sl, :CV], vstg[:, sl, :])

                # ---- lazy PE transposes q,k -> qT,kT [64, N] f32r ----
                qT = qkt_pool.tile([64, N], F32R, tag="qT")
                kT = qkt_pool.tile([64, N], F32R, tag="kT")

                def make_qkt(g, stg, dst):
                    def emit():
                        tp = ps_s.tile([64, 1024], F32, tag="s", name="tp")
                        for u in range(4):
                            t = 4 * g + u
                            nc.tensor.transpose(
                                tp[:, u * 128 : (u + 1) * 128],
                                stg[:, t, :],
                                ident[:],
                            )
                        nc.vector.tensor_copy(
                            dst[:, g * 512 : (g + 1) * 512], tp[:, :512]
                        )

                    return emit

                make_qkt(0, q_stg, qT)()
                make_qkt(0, k_stg, kT)()
                qk_pending = [
                    make_qkt(g, stg, dst)
                    for g in range(1, NT // 4)
                    for stg, dst in ((q_stg, qT), (k_stg, kT))
                ]
                qk_done = [0]

                def flush_qk(up_to_group):
                    while qk_done[0] < up_to_group and qk_pending:
                        qk_pending.pop(0)()
                        qk_pending.pop(0)()
                        qk_done[0] += 1

                # ---- main loop over q-chunks ----
                for c in range(NCHUNK):
                    flush_qk(min(c + 1, NT // 4 - 1))
                    njs = 4 * c + 4
                    o_ps = [
                        ps_o.tile([128, 2 * (CV + 1)], F32, tag="o01", name="o01"),
                        ps_o.tile([128, 2 * (CV + 1)], F32, tag="o23", name="o23"),
                    ]

                    carry_pv = [None]

                    def make_pv(c, p_t, ja, jb):
                        def emit():
                            for u, j in enumerate((ja, jb)):
                                base = 512 * u
                                t0 = max(0, j - 4 * c)
                                for i in range(t0, 4):
                                    # Two accumulation groups share each PSUM
                                    # bank. start=True pending-zeroes the WHOLE
                                    # 2KB bank, so only the first group (even i)
                                    # starts; the odd group's first write rides
                                    # the bank-wide pending-zero. Only the
                                    # last-finishing group (odd i) stops.
                                    nc.tensor.matmul(
                                        o_ps[i // 2][
                                            :,
                                            (i % 2) * (CV + 1) : (i % 2 + 1) * (CV + 1),
                                        ],
                                        p_t[:, base + 128 * i : base + 128 * (i + 1)],
                                        v_aug[:, j, :],
                                        start=(j == 0 and i % 2 == 0),
                                        stop=(i % 2 == 1 and j == 4 * c + i),
                                        skip_group_check=True,
                                    )

                        return emit

                    for u in range(njs // 2):
                        ja, jb = 2 * u, 2 * u + 1
                        s_ps = ps_s.tile([128, 1024], F32, tag="s", name="s_ps")
                        p_t = p_pool.tile([128, 1024], BF16, tag="p", name="p_t")
                        exts = []  # valid extent per j
                        for w, j in enumerate((ja, jb)):
                            base = 512 * w
                            t0 = max(0, j - 4 * c)
                            ext = 512 - 128 * t0     # valid q-columns
                            # streamed width: >=256 keeps f32r at 1 cyc/row;
                            # t0==1 streams full 512 so the pair's PSUM tile is
                            # fully written and one merged exp reads no stale.
                            se = 512 if t0 <= 1 else 256
                            nc.tensor.matmul(
                                s_ps[:, base + 512 - se : base + 512],
                                kT[:, 128 * j : 128 * (j + 1)],
                                qT[:, 512 * c + 512 - se : 512 * (c + 1)],
                                start=True,
                                stop=True,
                            )
                            exts.append(ext)
                        ea, eb = exts
                        if ea == 512 and eb >= 384:
                            # one activation over the fully-written tile; the
                            # non-causal overcompute region is never read.
                            nc.scalar.activation(
                                p_t[:, :1024],
                                s_ps[:, :1024],
                                mybir.ActivationFunctionType.Exp,
                                scale=SCALE,
                            )
                        else:
                            nc.scalar.activation(
                                p_t[:, 512 - ea : 512],
                                s_ps[:, 512 - ea : 512],
                                mybir.ActivationFunctionType.Exp,
                                scale=SCALE,
                            )
                            nc.scalar.activation(
                                p_t[:, 1024 - eb : 1024],
                                s_ps[:, 1024 - eb : 1024],
                                mybir.ActivationFunctionType.Exp,
                                scale=SCALE,
                            )
                        # strict-causal zeroing on the diagonal k-tiles
                        for w, j in enumerate((ja, jb)):
                            t0 = j - 4 * c
                            if 0 <= t0 <= 3:
                                sl = p_t[
                                    :, 512 * w + 128 * t0 : 512 * w + 128 * (t0 + 1)
                                ]
                                nc.vector.tensor_mul(sl, sl, mask_bf[:])
                        if carry_pv[0] is not None:
                            carry_pv[0]()
                        carry_pv[0] = make_pv(c, p_t, ja, jb)

                    carry_pv[0]()

                    # ---- normalize + store ----
                    for i in range(4):
                        sl = o_ps[i // 2][
                            :, (i % 2) * (CV + 1) : (i % 2 + 1) * (CV + 1)
                        ]
                        ssum = stats_pool.tile(
                            [128, 1], F32, tag=f"ss{i}", name="ssum"
                        )
                        nc.vector.tensor_scalar_add(
                            ssum[:], sl[:, CV : CV + 1], 1e-30
                        )
                        recip = stats_pool.tile(
                            [128, 1], F32, tag=f"rc{i}", name="recip"
                        )
                        nc.vector.reciprocal(recip[:], ssum[:])
                        o_sb = o_pool.tile(
                            [128, CV], F32, tag=f"ob{i}", name="o_sb"
                        )
                        nc.vector.tensor_scalar_mul(o_sb[:], sl[:, :CV], recip[:])
                        ti = 4 * c + i
                        nc.sync.dma_start(o[128 * ti : 128 * (ti + 1), :], o_sb[:])

            if repeats > 1:
                with tc.For_i(0, repeats, 1):
                    emit_body()
            else:
                emit_body()

    nc.compile()
    return nc


_NC_CACHE = None


def kernel(**inputs: np.ndarray) -> np.ndarray:
    global _NC_CACHE
    if _NC_CACHE is None:
        _NC_CACHE = build_kernel()
    nc = _NC_CACHE

    query = np.ascontiguousarray(inputs["query"], dtype=np.float32)
    key = np.ascontiguousarray(inputs["key"], dtype=np.float32)
    value = np.ascontiguousarray(inputs["value"], dtype=np.float32)

    in_maps = [
        {
            "q": query[b].reshape(N, CK),
            "k": key[b].reshape(N, CK),
            "v": value[b].reshape(N, CV),
        }
        for b in range(B)
    ]
    res = run_bass_kernel_spmd(nc, in_maps, list(range(B)))
    out = np.stack([res.results[b]["o"] for b in range(B)], axis=0)
    return out.reshape(B, H, W, CV)


def run_traced(inputs_np):
    """Run with NTFF tracing, return HW exec time in ns (max over cores)."""
    global _NC_CACHE
    if _NC_CACHE is None:
        _NC_CACHE = build_kernel()
    nc = _NC_CACHE
    query = inputs_np["query"].reshape(B, N, CK)
    key = inputs_np["key"].reshape(B, N, CK)
    value = inputs_np["value"].reshape(B, N, CV)
    in_maps = [
        {"q": query[b], "k": key[b], "v": value[b]} for b in range(B)
    ]
    res = run_bass_kernel_spmd(nc, in_maps, list(range(B)), trace=True)
    return res.exec_time_ns


if __name__ == "__main__":
    rng = np.random.default_rng(0)
    qq = rng.standard_normal((B, H, W, CK), dtype=np.float32)
    kk = rng.standard_normal((B, H, W, CK), dtype=np.float32)
    vv = rng.standard_normal((B, H, W, CV), dtype=np.float32)
    out = kernel(query=qq, key=kk, value=vv)
    print("out", out.shape, out.dtype, np.abs(out).mean())


# revision 14
# speedup vs baseline: 2.0180x; 1.1726x over previous
"""Trainium2 Bass kernel for strict-causal (pixelSNAIL) attention.

Problem: B=8, H=W=64 (N=4096), Ck=64, Cv=128, fp32.
    out[b] = softmax(mask(q@k^T/sqrt(Ck))) @ v   with strictly-causal mask
    (pixel i attends only to j < i; row 0 gets all-zero output).

Sharding: data-parallel over batch - one batch per NeuronCore, 8 cores.

Per-core algorithm (v3: transposed-score layout, no P transposes, bf16):
  - DVE-convert q,k to bf16; PE-transpose -> qT,kT [64, 4096] bf16.
  - For each q-chunk of 512 rows, loop over k-tiles j (causal extent),
    two j per PSUM tile pair:
      S^T[128k, q..] = kT_j^T @ qT_chunk   (bf16 matmul, PSUM, exact extent)
      P^T = exp(0.125*S^T)  ScalarE, PSUM->SBUF bf16 (valid region only)
      diagonal k-tile: strict-causal zeroing of P^T via DVE mask multiply
      O[128q, 129] += P^T_{j,i}^T @ [V_j | 1]  (bf16 matmul per q-tile i,
         PSUM accumulate over j; col 128 accumulates the softmax rowsum)
  - Normalize on DVE: recip = 1/(rowsum+eps); o_chunk = O * recip;
    one output DMA per chunk. Output lands in [q, v] layout directly.
"""

import os
import sys

sys.path.insert(0, "/opt/trn_rl_repo")

import numpy as np

import concourse.bass as bass
import concourse.bacc as bacc
import concourse.mybir as mybir
import concourse.tile as tile
from concourse.bass_utils import run_bass_kernel_spmd
from concourse.masks import make_identity

F32 = mybir.dt.float32
BF16 = mybir.dt.bfloat16

B, H, W, CK, CV = 8, 64, 64, 64, 128
N = H * W            # 4096
NT = N // 128        # 32 q-tiles / k-tiles
NCHUNK = N // 512    # 8 q-chunks
SCALE = 1.0 / np.sqrt(CK)


def build_kernel(repeats=1):
    nc = bacc.Bacc("TRN2", target_bir_lowering=False, debug=False, num_devices=8)

    q = nc.dram_tensor("q", [N, CK], F32, kind="ExternalInput").ap()
    k = nc.dram_tensor("k", [N, CK], F32, kind="ExternalInput").ap()
    v = nc.dram_tensor("v", [N, CV], F32, kind="ExternalInput").ap()
    o = nc.dram_tensor("o", [N, CV], F32, kind="ExternalOutput").ap()

    with tile.TileContext(nc) as tc:
        with (
            tc.tile_pool(name="const", bufs=1) as const_pool,
            tc.tile_pool(name="stage", bufs=1) as stage_pool,
            tc.tile_pool(name="qkT", bufs=1) as qkt_pool,
            tc.tile_pool(name="vsb", bufs=1) as v_pool,
            tc.tile_pool(name="p", bufs=3) as p_pool,
            tc.tile_pool(name="osb", bufs=2) as o_pool,
            tc.tile_pool(name="stats", bufs=8) as stats_pool,
            tc.tile_pool(name="ps_s", bufs=2, space="PSUM") as ps_s,
            tc.tile_pool(name="ps_o", bufs=2, space="PSUM") as ps_o,
        ):
            def emit_body():
                # ---- constants ----
                ident = const_pool.tile([128, 128], F32)
                make_identity(nc, ident[:])
                ident_bf = const_pool.tile([128, 128], BF16)
                nc.vector.tensor_copy(ident_bf[:], ident[:])
                # strict-causal keep-mask for diagonal tiles of P^T[k, q]:
                # 1.0 where k < q (partition < column), else 0.0
                mask_bf = const_pool.tile([128, 128], BF16)
                nc.gpsimd.memset(mask_bf[:], 1.0)
                nc.gpsimd.affine_select(
                    out=mask_bf[:],
                    in_=mask_bf[:],
                    compare_op=mybir.AluOpType.is_gt,  # keep 1 where q - k > 0
                    fill=0.0,
                    base=0,
                    pattern=[[1, 128]],
                    channel_multiplier=-1,
                )

                # ---- load q, k, v; convert to bf16; v_aug = [V | 1] ----
                q_stg = stage_pool.tile([128, NT, CK], F32, tag="q_stage")
                k_stg = stage_pool.tile([128, NT, CK], F32, tag="k_stage")
                vstg = stage_pool.tile([128, NT, CV], F32, tag="v_stage")
                q_bf = stage_pool.tile([128, NT, CK], BF16, tag="q_bf")
                k_bf = stage_pool.tile([128, NT, CK], BF16, tag="k_bf")
                v_aug = v_pool.tile([128, NT, CV + 1], BF16)

                nc.vector.memset(v_aug[:, :, CV], 1.0)

                q_r = q.rearrange("(t p) c -> p t c", p=128)
                k_r = k.rearrange("(t p) c -> p t c", p=128)
                v_r = v.rearrange("(t p) c -> p t c", p=128)
                for d in range(8):
                    sl = slice(4 * d, 4 * (d + 1))
                    nc.sync.dma_start(k_stg[:, sl, :], k_r[:, sl, :])
                    nc.sync.dma_start(q_stg[:, sl, :], q_r[:, sl, :])
                    nc.sync.dma_start(vstg[:, sl, :], v_r[:, sl, :])
                    nc.vector.tensor_copy(k_bf[:, sl, :], k_stg[:, sl, :])
                    nc.vector.tensor_copy(q_bf[:, sl, :], q_stg[:, sl, :])
                    nc.vector.tensor_copy(v_aug[:, sl, :CV], vstg[:, sl, :])

                # ---- lazy PE transposes q,k -> qT,kT [64, N] bf16 ----
                qT = qkt_pool.tile([64, N], BF16, tag="qT")
                kT = qkt_pool.tile([64, N], BF16, tag="kT")

                def make_qkt(g, stg, dst):
                    def emit():
                        tp = ps_s.tile([64, 2048], BF16, tag="s", name="tp")
                        for u in range(4):
                            t = 4 * g + u
                            nc.tensor.transpose(
                                tp[:, u * 128 : (u + 1) * 128],
                                stg[:, t, :],
                                ident_bf[:],
                            )
                        nc.vector.tensor_copy(
                            dst[:, g * 512 : (g + 1) * 512], tp[:, :512]
                        )

                    return emit

                make_qkt(0, k_bf, kT)()
                make_qkt(0, q_bf, qT)()
                qk_pending = [
                    make_qkt(g, stg, dst)
                    for g in range(1, NT // 4)
                    for stg, dst in ((q_bf, qT), (k_bf, kT))
                ]
                qk_done = [0]

                def flush_qk(up_to_group):
                    while qk_done[0] < up_to_group and qk_pending:
                        qk_pending.pop(0)()
                        qk_pending.pop(0)()
                        qk_done[0] += 1

                # ---- main loop over q-chunks ----
                for c in range(NCHUNK):
                    flush_qk(min(c + 1, NT // 4 - 1))
                    njs = 4 * c + 4
                    o_ps = [
                        ps_o.tile([128, 2 * (CV + 1)], F32, tag="o01", name="o01"),
                        ps_o.tile([128, 2 * (CV + 1)], F32, tag="o23", name="o23"),
                    ]

                    carry_pv = [None]

                    def make_pv(c, p_t, ja, jb):
                        def emit():
                            for u, j in enumerate((ja, jb)):
                                base = 512 * u
                                t0 = max(0, j - 4 * c)
                                for i in range(t0, 4):
                                    # Two accumulation groups share each PSUM
                                    # bank. start=True pending-zeroes the WHOLE
                                    # 2KB bank, so only the first group (even i)
                                    # starts; the odd group's first write rides
                                    # the bank-wide pending-zero. Only the
                                    # last-finishing group (odd i) stops.
                                    nc.tensor.matmul(
                                        o_ps[i // 2][
                                            :,
                                            (i % 2) * (CV + 1) : (i % 2 + 1) * (CV + 1),
                                        ],
                                        p_t[:, base + 128 * i : base + 128 * (i + 1)],
                                        v_aug[:, j, :],
                                        start=(j == 0 and i % 2 == 0),
                                        stop=(i % 2 == 1 and j == 4 * c + i),
                                        skip_group_check=True,
                                    )

                        return emit

                    for u in range(njs // 2):
                        ja, jb = 2 * u, 2 * u + 1
                        s_ps = ps_s.tile([128, 1024], F32, tag="s", name="s_ps")
                        p_t = p_pool.tile([128, 1024], BF16, tag="p", name="p_t")
                        exts = []
                        for w, j in enumerate((ja, jb)):
                            base = 512 * w
                            t0 = max(0, j - 4 * c)
                            ext = 512 - 128 * t0     # valid q-columns
                            nc.tensor.matmul(
                                s_ps[:, base + 512 - ext : base + 512],
                                kT[:, 128 * j : 128 * (j + 1)],
                                qT[:, 512 * c + 512 - ext : 512 * (c + 1)],
                                start=True,
                                stop=True,
                            )
                            exts.append(ext)
                        ea, eb = exts
                        if ea == 512 and eb == 512:
                            nc.scalar.activation(
                                p_t[:, :1024],
                                s_ps[:, :1024],
                                mybir.ActivationFunctionType.Exp,
                                scale=SCALE,
                            )
                        else:
                            nc.scalar.activation(
                                p_t[:, 512 - ea : 512],
                                s_ps[:, 512 - ea : 512],
                                mybir.ActivationFunctionType.Exp,
                                scale=SCALE,
                            )
                            nc.scalar.activation(
                                p_t[:, 1024 - eb : 1024],
                                s_ps[:, 1024 - eb : 1024],
                                mybir.ActivationFunctionType.Exp,
                                scale=SCALE,
                            )
                        # strict-causal zeroing on the diagonal k-tiles
                        for w, j in enumerate((ja, jb)):
                            t0 = j - 4 * c
                            if 0 <= t0 <= 3:
                                sl = p_t[
                                    :, 512 * w + 128 * t0 : 512 * w + 128 * (t0 + 1)
                                ]
                                nc.vector.tensor_mul(sl, sl, mask_bf[:])
                        if carry_pv[0] is not None:
                            carry_pv[0]()
                        carry_pv[0] = make_pv(c, p_t, ja, jb)

                    carry_pv[0]()

                    # ---- normalize + store (one DMA per chunk) ----
                    o_ch = o_pool.tile([128, 4, CV], F32, tag="o_ch", name="o_ch")
                    for i in range(4):
                        sl = o_ps[i // 2][
                            :, (i % 2) * (CV + 1) : (i % 2 + 1) * (CV + 1)
                        ]
                        ssum = stats_pool.tile(
                            [128, 1], F32, tag=f"ss{i}", name="ssum"
                        )
                        nc.vector.tensor_scalar_add(
                            ssum[:], sl[:, CV : CV + 1], 1e-30
                        )
                        recip = stats_pool.tile(
                            [128, 1], F32, tag=f"rc{i}", name="recip"
                        )
                        nc.vector.reciprocal(recip[:], ssum[:])
                        nc.vector.tensor_scalar_mul(
                            o_ch[:, i, :], sl[:, :CV], recip[:]
                        )
                    nc.sync.dma_start(
                        o[512 * c : 512 * (c + 1), :].rearrange(
                            "(t p) c -> p t c", p=128
                        ),
                        o_ch[:],
                    )

            if repeats > 1:
                with tc.For_i(0, repeats, 1):
                    emit_body()
            else:
                emit_body()

    nc.compile()
    return nc


_NC_CACHE = None


def kernel(**inputs: np.ndarray) -> np.ndarray:
    global _NC_CACHE
    if _NC_CACHE is None:
        _NC_CACHE = build_kernel()
    nc = _NC_CACHE

    query = np.ascontiguousarray(inputs["query"], dtype=np.float32)
    key = np.ascontiguousarray(inputs["key"], dtype=np.float32)
    value = np.ascontiguousarray(inputs["value"], dtype=np.float32)

    in_maps = [
        {
            "q": query[b].reshape(N, CK),
            "k": key[b].reshape(N, CK),
            "v": value[b].reshape(N, CV),
        }
        for b in range(B)
    ]
    res = run_bass_kernel_spmd(nc, in_maps, list(range(B)))
    out = np.stack([res.results[b]["o"] for b in range(B)], axis=0)
    return out.reshape(B, H, W, CV)


def run_traced(inputs_np):
    """Run with NTFF tracing, return HW exec time in ns (max over cores)."""
    global _NC_CACHE
    if _NC_CACHE is None:
        _NC_CACHE = build_kernel()
    nc = _NC_CACHE
    query = inputs_np["query"].reshape(B, N, CK)
    key = inputs_np["key"].reshape(B, N, CK)
    value = inputs_np["value"].reshape(B, N, CV)
    in_maps = [
        {"q": query[b], "k": key[b], "v": value[b]} for b in range(B)
    ]
    res = run_bass_kernel_spmd(nc, in_maps, list(range(B)), trace=True)
    return res.exec_time_ns


if __name__ == "__main__":
    rng = np.random.default_rng(0)
    qq = rng.standard_normal((B, H, W, CK), dtype=np.float32)
    kk = rng.standard_normal((B, H, W, CK), dtype=np.float32)
    vv = rng.standard_normal((B, H, W, CV), dtype=np.float32)
    out = kernel(query=qq, key=kk, value=vv)
    print("out", out.shape, out.dtype, np.abs(out).mean())
